# revision 14
# baseline (speedup 1.0000x reference)
"""v4: per-super-block deferred pooling, engine-balanced from measured rates.

Measured DVE rates (ns/row, row = 128 lanes): TENSOR_REDUCE 1.13 (no fast
mode), TENSOR_TENSOR packed-bf16 0.49 + ~170 fixed, TENSOR_SCALAR from PSUM
~1.04 + ~130 fixed, ACT 1.06 + ~65, GPS TT add 1.9 + 95 launch, PE accum
matmul ~123 ns ldweights + latency (amortized via SUPER-block free dims).

Structure per super-block (SUPER x BLK lanes):
  - blocks stream: DMA x, matmul waves -> PSUM, evac relu(u+b) -> r_super
    (ACT cols [0:ACT_COLS], DVE tensor_scalar the rest)
  - pooling for super i runs during super i+1 (so evacs are complete):
      max: DVE TT halves-tree per chunk (packed bf16, 2x mode)
      sum: per super engine pattern: PE (16 accumulating identity matmuls,
           chunk-interleaved chains), GPS add-tree, or DVE add-tree
"""
import sys

sys.path.insert(0, "/opt/trn_rl_repo")

import numpy as np
import ml_dtypes

import concourse.bass as bass
import concourse.bacc as bacc
import concourse.tile as tile
from concourse import mybir
from concourse.bass_utils import run_bass_kernel_spmd

N_CORES = 8
IN_DIM = 128
OUT_DIM = 512
N_OBS = 25000
M_LANES = 400000
GS = 16
M_C = M_LANES // N_CORES
G_C = N_OBS // N_CORES
N_CHUNK = OUT_DIM // 128
BLK = 1024
SUPER = 6

# --- engine-balance tunables -------------------------------------------------
ACT_COLS = 848                # evac cols per span on ACT; rest on DVE
SUM_PATTERN = ("pe", "gps", "pe", "gps", "pe", "gps", "pe", "dve", "gps")

_compiled = {}


def _blocks(m_c, blk):
    out = []
    s = 0
    while s < m_c:
        out.append((s, min(blk, m_c - s)))
        s += blk
    return out


def _build(m_c: int = M_C) -> bass.Bass:
    nc = bacc.Bacc(None, target_bir_lowering=False)
    f32 = mybir.dt.float32
    bf16 = mybir.dt.bfloat16
    g_c = m_c // GS

    xth_d = nc.dram_tensor("xth", [IN_DIM, m_c], bf16, kind="ExternalInput")
    wth_d = nc.dram_tensor("wth", [IN_DIM, OUT_DIM], bf16, kind="ExternalInput")
    bsc_d = nc.dram_tensor("bsc", [128, N_CHUNK], f32, kind="ExternalInput")
    eye_d = nc.dram_tensor("eye", [128, 128], bf16, kind="ExternalInput")
    omax_d = nc.dram_tensor("omax", [OUT_DIM, g_c], bf16, kind="ExternalOutput")
    osum_d = nc.dram_tensor("osum", [OUT_DIM, g_c], bf16, kind="ExternalOutput")

    blocks = _blocks(m_c, BLK)
    nblk = len(blocks)
    n_super = (nblk + SUPER - 1) // SUPER

    def super_range(isup):
        """(first block idx, lane count, group start, group count)"""
        b0 = isup * SUPER
        bl = blocks[b0 : b0 + SUPER]
        lanes = sum(b[1] for b in bl)
        return b0, lanes, blocks[b0][0] // GS, lanes // GS

    def sum_engine(isup):
        return SUM_PATTERN[isup % len(SUM_PATTERN)]

    with nc.allow_low_precision("pooled outputs are bf16 by design"), \
            tile.TileContext(nc) as tc:
        with (
            tc.tile_pool(name="singles", bufs=1) as singles,
            tc.tile_pool(name="xin", bufs=3) as xin,
            tc.tile_pool(name="rsb", bufs=2) as rsb,
            tc.tile_pool(name="gtmp", bufs=1) as gtmp,
            tc.tile_pool(name="acc", bufs=1) as accp,
            tc.tile_pool(name="psum_u", bufs=2, space="PSUM") as psum_u,
            tc.tile_pool(name="psum_s", bufs=1, space="PSUM") as psum_s,
        ):
            wth_sb = singles.tile([IN_DIM, OUT_DIM], bf16)
            nc.sync.dma_start(out=wth_sb, in_=wth_d[:, :])
            eye_sb = singles.tile([128, 128], bf16)
            nc.sync.dma_start(out=eye_sb, in_=eye_d[:, :])
            bsc_sb = singles.tile([128, N_CHUNK], f32)
            nc.sync.dma_start(out=bsc_sb, in_=bsc_d[:, :])

            maxp_sb = accp.tile([128, N_CHUNK, g_c], bf16)
            sump_sb = accp.tile([128, N_CHUNK, g_c], bf16)

            warm_sb = singles.tile([128, 2], f32)
            nc.vector.memset(warm_sb, 0.0)
            nc.scalar.activation(
                out=warm_sb, in_=warm_sb,
                func=mybir.ActivationFunctionType.Relu, bias=0.0, scale=1.0,
            )

            flush_from = 0
            r_super = None
            prev = None  # (isup, r_super, lanes, g0, gcount)

            def emit_max_tree(r_sup, lanes, g0, gcnt):
                r4 = r_sup[0:128, :, 0:lanes].rearrange(
                    "p c (g s) -> p c g s", s=GS
                )
                for c in range(N_CHUNK):
                    t8 = gtmp.tile([128, SUPER * BLK // GS, 8], bf16, tag="m8")
                    nc.vector.tensor_tensor(
                        out=t8[:, 0:gcnt, :],
                        in0=r4[:, c, :, 0:8], in1=r4[:, c, :, 8:16],
                        op=mybir.AluOpType.max,
                    )
                    t4 = gtmp.tile([128, SUPER * BLK // GS, 4], bf16, tag="m4")
                    nc.vector.tensor_tensor(
                        out=t4[:, 0:gcnt, :],
                        in0=t8[:, 0:gcnt, 0:4], in1=t8[:, 0:gcnt, 4:8],
                        op=mybir.AluOpType.max,
                    )
                    t2 = gtmp.tile([128, SUPER * BLK // GS, 2], bf16, tag="m2")
                    nc.vector.tensor_tensor(
                        out=t2[:, 0:gcnt, :],
                        in0=t4[:, 0:gcnt, 0:2], in1=t4[:, 0:gcnt, 2:4],
                        op=mybir.AluOpType.max,
                    )
                    nc.vector.tensor_tensor(
                        out=maxp_sb[:, c, g0 : g0 + gcnt],
                        in0=t2[:, 0:gcnt, 0], in1=t2[:, 0:gcnt, 1],
                        op=mybir.AluOpType.max,
                    )

            def emit_sum_tree_dve(r_sup, lanes, g0, gcnt):
                r4 = r_sup[0:128, :, 0:lanes].rearrange(
                    "p c (g s) -> p c g s", s=GS
                )
                for c in range(N_CHUNK):
                    t8 = gtmp.tile([128, SUPER * BLK // GS, 8], bf16, tag="s8")
                    nc.vector.tensor_tensor(
                        out=t8[:, 0:gcnt, :],
                        in0=r4[:, c, :, 0:8], in1=r4[:, c, :, 8:16],
                        op=mybir.AluOpType.add,
                    )
                    t4 = gtmp.tile([128, SUPER * BLK // GS, 4], bf16, tag="s4")
                    nc.vector.tensor_tensor(
                        out=t4[:, 0:gcnt, :],
                        in0=t8[:, 0:gcnt, 0:4], in1=t8[:, 0:gcnt, 4:8],
                        op=mybir.AluOpType.add,
                    )
                    t2 = gtmp.tile([128, SUPER * BLK // GS, 2], bf16, tag="s2")
                    nc.vector.tensor_tensor(
                        out=t2[:, 0:gcnt, :],
                        in0=t4[:, 0:gcnt, 0:2], in1=t4[:, 0:gcnt, 2:4],
                        op=mybir.AluOpType.add,
                    )
                    nc.vector.tensor_tensor(
                        out=sump_sb[:, c, g0 : g0 + gcnt],
                        in0=t2[:, 0:gcnt, 0], in1=t2[:, 0:gcnt, 1],
                        op=mybir.AluOpType.add,
                    )

            def emit_sum_tree_gps(r_sup, lanes, g0, gcnt):
                r4 = r_sup[0:128, :, 0:lanes].rearrange(
                    "p c (g s) -> p c g s", s=GS
                )
                for c in range(N_CHUNK):
                    t8 = gtmp.tile([128, SUPER * BLK // GS, 8], bf16, tag="g8")
                    nc.gpsimd.tensor_tensor(
                        out=t8[:, 0:gcnt, :],
                        in0=r4[:, c, :, 0:8], in1=r4[:, c, :, 8:16],
                        op=mybir.AluOpType.add,
                    )
                    t4 = gtmp.tile([128, SUPER * BLK // GS, 4], bf16, tag="g4")
                    nc.gpsimd.tensor_tensor(
                        out=t4[:, 0:gcnt, :],
                        in0=t8[:, 0:gcnt, 0:4], in1=t8[:, 0:gcnt, 4:8],
                        op=mybir.AluOpType.add,
                    )
                    t2 = gtmp.tile([128, SUPER * BLK // GS, 2], bf16, tag="g2")
                    nc.gpsimd.tensor_tensor(
                        out=t2[:, 0:gcnt, :],
                        in0=t4[:, 0:gcnt, 0:2], in1=t4[:, 0:gcnt, 2:4],
                        op=mybir.AluOpType.add,
                    )
                    nc.gpsimd.tensor_tensor(
                        out=sump_sb[:, c, g0 : g0 + gcnt],
                        in0=t2[:, 0:gcnt, 0], in1=t2[:, 0:gcnt, 1],
                        op=mybir.AluOpType.add,
                    )

            def emit_sum_pe(r_sup, lanes, g0, gcnt):
                r4 = r_sup[0:128, :, 0:lanes].rearrange(
                    "p c (g s) -> p c g s", s=GS
                )
                # 512 f32 per chunk = exactly one PSUM bank, so each chunk's
                # accumulation region never crosses a bank boundary
                ps_tile = psum_s.tile([128, N_CHUNK, 512], f32, tag="ps")
                for s in range(GS):
                    for c in range(N_CHUNK):
                        nc.tensor.matmul(
                            ps_tile[:, c, 0:gcnt],
                            eye_sb[:, :],
                            r4[:, c, :, s],
                            start=(s == 0), stop=(s == GS - 1),
                        )
                nc.scalar.activation(
                    out=sump_sb[:, :, g0 : g0 + gcnt],
                    in_=ps_tile[:, :, 0:gcnt],
                    func=mybir.ActivationFunctionType.Copy,
                    bias=0.0, scale=1.0,
                )

            def emit_pools(p):
                pisup, r_sup, lanes, g0, gcnt = p
                emit_max_tree(r_sup, lanes, g0, gcnt)
                eng = sum_engine(pisup)
                if eng == "pe":
                    emit_sum_pe(r_sup, lanes, g0, gcnt)
                elif eng == "gps":
                    emit_sum_tree_gps(r_sup, lanes, g0, gcnt)
                else:
                    emit_sum_tree_dve(r_sup, lanes, g0, gcnt)

            for ib, (l0, lb) in enumerate(blocks):
                isup = ib // SUPER
                boff = ib % SUPER

                if boff == 0:
                    r_super = rsb.tile([128, N_CHUNK, SUPER * BLK], bf16,
                                       tag="r")

                xth_sb = xin.tile([IN_DIM, BLK], bf16, tag="xth")
                nc.sync.dma_start(out=xth_sb[:, :lb], in_=xth_d[:, l0 : l0 + lb])

                n_wave = (lb + 511) // 512
                for c in range(N_CHUNK):
                    enc_ps = psum_u.tile([128, BLK], f32, tag="enc")
                    for w in range(n_wave):
                        w0 = w * 512
                        lw = min(512, lb - w0)
                        nc.tensor.matmul(
                            enc_ps[:, w0 : w0 + lw],
                            wth_sb[:, c * 128 : (c + 1) * 128],
                            xth_sb[:, w0 : w0 + lw],
                            start=True, stop=True,
                        )

                    # pools of the previous super-block, one block of slack
                    # after its last evac
                    if prev is not None and boff == 1 and c == 1:
                        emit_pools(prev)
                        prev = None

                    rc = r_super[:, c, boff * BLK : boff * BLK + lb]
                    sa = min(ACT_COLS, lb)
                    nc.scalar.activation(
                        out=rc[0:128, 0:sa],
                        in_=enc_ps[:, 0:sa],
                        func=mybir.ActivationFunctionType.Relu,
                        bias=bsc_sb[:, c : c + 1],
                        scale=1.0,
                    )
                    if lb > sa:
                        nc.vector.tensor_scalar(
                            out=rc[0:128, sa:lb],
                            in0=enc_ps[:, sa:lb],
                            scalar1=bsc_sb[:, c : c + 1],
                            scalar2=0.0,
                            op0=mybir.AluOpType.add,
                            op1=mybir.AluOpType.max,
                        )

                if boff == SUPER - 1 or ib == nblk - 1:
                    if prev is not None:
                        # single-block last super: consume the pending super
                        # before overwriting (its evacs finished long ago)
                        emit_pools(prev)
                    _b0, lanes, sg0, sgcnt = super_range(isup)
                    prev = (isup, r_super, lanes, sg0, sgcnt)

                # flush pooled outputs whose writers are already emitted
                # (through super isup-1) at mid-super points
                if boff == 3 and isup in (2, 4, 6) :
                    done = super_range(isup)[2]
                    if done > flush_from:
                        r0, r1 = flush_from, done
                        flush_from = done
                        for c in range(N_CHUNK):
                            nc.sync.dma_start(
                                out=omax_d[c * 128 : (c + 1) * 128, r0:r1],
                                in_=maxp_sb[:, c, r0:r1],
                            )
                            nc.sync.dma_start(
                                out=osum_d[c * 128 : (c + 1) * 128, r0:r1],
                                in_=sump_sb[:, c, r0:r1],
                            )

            if prev is not None:
                emit_pools(prev)
                prev = None

            r0, r1 = flush_from, g_c
            for c in range(N_CHUNK):
                nc.sync.dma_start(
                    out=omax_d[c * 128 : (c + 1) * 128, r0:r1],
                    in_=maxp_sb[:, c, r0:r1],
                )
                nc.sync.dma_start(
                    out=osum_d[c * 128 : (c + 1) * 128, r0:r1],
                    in_=sump_sb[:, c, r0:r1],
                )

    nc.compile()
    return nc


def _get_nc() -> bass.Bass:
    if "k" not in _compiled:
        _compiled["k"] = _build()
    return _compiled["k"]


def _host_prep(lane_encoding, W, b):
    bf = ml_dtypes.bfloat16
    xT = np.ascontiguousarray(lane_encoding.T)
    xh = xT.astype(bf)
    wh = np.ascontiguousarray(W.T).astype(bf)
    bsc = np.ascontiguousarray(b.reshape(N_CHUNK, 128).T.astype(np.float32))
    eye = np.eye(128, dtype=bf)

    in_maps = []
    for c in range(N_CORES):
        sl = slice(c * M_C, (c + 1) * M_C)
        in_maps.append({
            "xth": np.ascontiguousarray(xh[:, sl]),
            "wth": wh, "bsc": bsc, "eye": eye,
        })
    return in_maps


def _run(lane_encoding, W, b, trace: bool = False):
    nc = _get_nc()
    in_maps = _host_prep(lane_encoding, W, b)
    try:
        res = run_bass_kernel_spmd(
            nc, in_maps, core_ids=list(range(N_CORES)), trace=trace
        )
    except Exception:
        res = run_bass_kernel_spmd(
            nc, in_maps, core_ids=list(range(N_CORES)), trace=trace
        )
    out = np.empty((N_OBS, 2 * OUT_DIM), dtype=np.float32)
    for c in range(N_CORES):
        gsl = slice(c * G_C, (c + 1) * G_C)
        out[gsl, :OUT_DIM] = res.results[c]["omax"].T.astype(np.float32)
        out[gsl, OUT_DIM:] = res.results[c]["osum"].T.astype(np.float32) / GS
    return out, res


def kernel(obs_encoding, lane_encoding, same_obs_mask, W, b):
    out, _ = _run(
        np.asarray(lane_encoding, dtype=np.float32),
        np.asarray(W, dtype=np.float32),
        np.asarray(b, dtype=np.float32),
    )
    return out


# revision 15
# speedup vs baseline: 1.5042x; 1.5042x over previous
"""Trainium2 Bass kernel for AttentionalAggregation-style GNN pooling.

reference math:
    enc  = relu(lane_encoding @ W.T + b)            # [M=400000, 512]
    maxp = segment_max(enc, seg)                    # [N=25000, 512], 16 lanes/group
    avgp = segment_mean(enc, seg)                   # [N=25000, 512]
    out  = concat([maxp, avgp], axis=1)             # [N, 1024]

Strategy (8 NeuronCores, data-parallel over lanes; each core owns whole
groups). Streaming per-block structure (keeps every engine overlapped):
  - Host pre-transposes x -> XT [128, M] bf16; single-pass bf16 matmul
    (the 2e-2 gate leaves ~5x margin over bf16 rounding noise).
  - relu(u + b) fused into the PSUM->SBUF evacuation on ACT, bf16 out.
  - max pool (DVE-only capability): per-span TT halves-tree on packed bf16
    (packed TENSOR_TENSOR hits the 2x DVE mode; TENSOR_REDUCE never does).
  - sum pool: a fraction of spans fully on the GPSIMD add-tree; the rest
    split DVE (level 1) + GPSIMD (levels 2-4) to balance both engines.
  - Pooled outputs stay transposed bf16 [512, G]; host converts to fp32
    and applies the /16 mean divide.
"""
import sys

sys.path.insert(0, "/opt/trn_rl_repo")

import numpy as np
import ml_dtypes

import concourse.bass as bass
import concourse.bacc as bacc
import concourse.tile as tile
from concourse import mybir
from concourse.bass_utils import run_bass_kernel_spmd

N_CORES = 8
IN_DIM = 128
OUT_DIM = 512
N_OBS = 25000
M_LANES = 400000
GS = 16
M_C = M_LANES // N_CORES
G_C = N_OBS // N_CORES
N_CHUNK = OUT_DIM // 128
BLK = 1536

# --- engine-balance tunables -------------------------------------------------
# sum routing per span index: span % 5 < GPS_FULL_OF5 -> full GPS tree;
# otherwise split: DVE does level 1, GPS levels 2-4.
GPS_FULL_OF5 = 1
# max pool: True = TT halves-tree (packed 2x), False = plain reduce
MAX_TREE = True

_compiled = {}


def _blocks(m_c, blk):
    out = []
    s = 0
    while s < m_c:
        out.append((s, min(blk, m_c - s)))
        s += blk
    return out


def _build(m_c: int = M_C) -> bass.Bass:
    nc = bacc.Bacc(None, target_bir_lowering=False)
    f32 = mybir.dt.float32
    bf16 = mybir.dt.bfloat16
    g_c = m_c // GS
    gb_max = BLK // GS

    xth_d = nc.dram_tensor("xth", [IN_DIM, m_c], bf16, kind="ExternalInput")
    wth_d = nc.dram_tensor("wth", [IN_DIM, OUT_DIM], bf16, kind="ExternalInput")
    bsc_d = nc.dram_tensor("bsc", [128, N_CHUNK], f32, kind="ExternalInput")
    omax_d = nc.dram_tensor("omax", [OUT_DIM, g_c], bf16, kind="ExternalOutput")
    osum_d = nc.dram_tensor("osum", [OUT_DIM, g_c], bf16, kind="ExternalOutput")

    blocks = _blocks(m_c, BLK)
    nblk = len(blocks)

    with nc.allow_low_precision("pooled outputs are bf16 by design"), \
            tile.TileContext(nc) as tc:
        with (
            tc.tile_pool(name="singles", bufs=1) as singles,
            tc.tile_pool(name="xin", bufs=3) as xin,
            tc.tile_pool(name="rsb", bufs=3) as rsb,
            tc.tile_pool(name="mt", bufs=2) as mtp,
            tc.tile_pool(name="st", bufs=2) as stp,
            tc.tile_pool(name="acc", bufs=1) as accp,
            tc.tile_pool(name="psum", bufs=2, space="PSUM") as psum,
        ):
            wth_sb = singles.tile([IN_DIM, OUT_DIM], bf16)
            nc.sync.dma_start(out=wth_sb, in_=wth_d[:, :])
            bsc_sb = singles.tile([128, N_CHUNK], f32)
            nc.sync.dma_start(out=bsc_sb, in_=bsc_d[:, :])

            maxp_sb = accp.tile([128, N_CHUNK, g_c], bf16)
            sump_sb = accp.tile([128, N_CHUNK, g_c], bf16)

            warm_sb = singles.tile([128, 2], f32)
            nc.vector.memset(warm_sb, 0.0)
            nc.scalar.activation(
                out=warm_sb, in_=warm_sb,
                func=mybir.ActivationFunctionType.Relu, bias=0.0, scale=1.0,
            )

            flush_at = {nblk // 4, nblk // 2, (3 * nblk) // 4, nblk - 1}
            flush_from = 0

            for ib, (l0, lb) in enumerate(blocks):
                gb = lb // GS
                g0 = l0 // GS

                xth_sb = xin.tile([IN_DIM, BLK], bf16, tag="xth")
                nc.sync.dma_start(out=xth_sb[:, :lb], in_=xth_d[:, l0 : l0 + lb])

                r_sb = rsb.tile([128, N_CHUNK, BLK], bf16, tag="r")

                n_wave = (lb + 511) // 512
                for c in range(N_CHUNK):
                    enc_ps = psum.tile([128, BLK], f32, tag="enc")
                    for w in range(n_wave):
                        w0 = w * 512
                        lw = min(512, lb - w0)
                        nc.tensor.matmul(
                            enc_ps[:, w0 : w0 + lw],
                            wth_sb[:, c * 128 : (c + 1) * 128],
                            xth_sb[:, w0 : w0 + lw],
                            start=True, stop=True,
                        )

                    rc = r_sb[:, c, :]
                    nc.scalar.activation(
                        out=rc[0:128, 0:lb],
                        in_=enc_ps[:, 0:lb],
                        func=mybir.ActivationFunctionType.Relu,
                        bias=bsc_sb[:, c : c + 1],
                        scale=1.0,
                    )

                    r3 = rc[0:128, 0:lb].rearrange("p (g s) -> p g s", s=GS)
                    span = ib * N_CHUNK + c

                    # ---- max pool (DVE halves-tree, packed bf16 2x) ----
                    if MAX_TREE:
                        m8 = mtp.tile([128, gb_max, 8], bf16, tag="m8")
                        nc.vector.tensor_tensor(
                            out=m8[:, 0:gb, :],
                            in0=r3[:, :, 0:8], in1=r3[:, :, 8:16],
                            op=mybir.AluOpType.max,
                        )
                        m4 = mtp.tile([128, gb_max, 4], bf16, tag="m4")
                        nc.vector.tensor_tensor(
                            out=m4[:, 0:gb, :],
                            in0=m8[:, 0:gb, 0:4], in1=m8[:, 0:gb, 4:8],
                            op=mybir.AluOpType.max,
                        )
                        m2 = mtp.tile([128, gb_max, 2], bf16, tag="m2")
                        nc.vector.tensor_tensor(
                            out=m2[:, 0:gb, :],
                            in0=m4[:, 0:gb, 0:2], in1=m4[:, 0:gb, 2:4],
                            op=mybir.AluOpType.max,
                        )
                        nc.vector.tensor_tensor(
                            out=maxp_sb[:, c, g0 : g0 + gb],
                            in0=m2[:, 0:gb, 0], in1=m2[:, 0:gb, 1],
                            op=mybir.AluOpType.max,
                        )
                    else:
                        nc.vector.reduce_max(
                            out=maxp_sb[:, c, g0 : g0 + gb],
                            in_=r3, axis=mybir.AxisListType.X,
                        )

                    # ---- sum pool ----
                    s8 = stp.tile([128, gb_max, 8], bf16, tag="s8")
                    if span % 5 < GPS_FULL_OF5:
                        nc.gpsimd.tensor_tensor(
                            out=s8[:, 0:gb, :],
                            in0=r3[:, :, 0:8], in1=r3[:, :, 8:16],
                            op=mybir.AluOpType.add,
                        )
                    else:
                        nc.vector.tensor_tensor(
                            out=s8[:, 0:gb, :],
                            in0=r3[:, :, 0:8], in1=r3[:, :, 8:16],
                            op=mybir.AluOpType.add,
                        )
                    s4 = stp.tile([128, gb_max, 4], bf16, tag="s4")
                    nc.gpsimd.tensor_tensor(
                        out=s4[:, 0:gb, :],
                        in0=s8[:, 0:gb, 0:4], in1=s8[:, 0:gb, 4:8],
                        op=mybir.AluOpType.add,
                    )
                    s2 = stp.tile([128, gb_max, 2], bf16, tag="s2")
                    nc.gpsimd.tensor_tensor(
                        out=s2[:, 0:gb, :],
                        in0=s4[:, 0:gb, 0:2], in1=s4[:, 0:gb, 2:4],
                        op=mybir.AluOpType.add,
                    )
                    nc.gpsimd.tensor_tensor(
                        out=sump_sb[:, c, g0 : g0 + gb],
                        in0=s2[:, 0:gb, 0], in1=s2[:, 0:gb, 1],
                        op=mybir.AluOpType.add,
                    )

                if ib in flush_at:
                    done = g0 + gb
                    if done > flush_from:
                        r0, r1 = flush_from, done
                        flush_from = done
                        for c in range(N_CHUNK):
                            nc.sync.dma_start(
                                out=omax_d[c * 128 : (c + 1) * 128, r0:r1],
                                in_=maxp_sb[:, c, r0:r1],
                            )
                            nc.sync.dma_start(
                                out=osum_d[c * 128 : (c + 1) * 128, r0:r1],
                                in_=sump_sb[:, c, r0:r1],
                            )

    nc.compile()
    return nc


def _get_nc() -> bass.Bass:
    if "k" not in _compiled:
        _compiled["k"] = _build()
    return _compiled["k"]


def _host_prep(lane_encoding, W, b):
    bf = ml_dtypes.bfloat16
    xT = np.ascontiguousarray(lane_encoding.T)
    xh = xT.astype(bf)
    wh = np.ascontiguousarray(W.T).astype(bf)
    bsc = np.ascontiguousarray(b.reshape(N_CHUNK, 128).T.astype(np.float32))

    in_maps = []
    for c in range(N_CORES):
        sl = slice(c * M_C, (c + 1) * M_C)
        in_maps.append({
            "xth": np.ascontiguousarray(xh[:, sl]),
            "wth": wh, "bsc": bsc,
        })
    return in_maps


def _run(lane_encoding, W, b, trace: bool = False):
    nc = _get_nc()
    in_maps = _host_prep(lane_encoding, W, b)
    try:
        res = run_bass_kernel_spmd(
            nc, in_maps, core_ids=list(range(N_CORES)), trace=trace
        )
    except Exception:
        res = run_bass_kernel_spmd(
            nc, in_maps, core_ids=list(range(N_CORES)), trace=trace
        )
    out = np.empty((N_OBS, 2 * OUT_DIM), dtype=np.float32)
    for c in range(N_CORES):
        gsl = slice(c * G_C, (c + 1) * G_C)
        out[gsl, :OUT_DIM] = res.results[c]["omax"].T.astype(np.float32)
        out[gsl, OUT_DIM:] = res.results[c]["osum"].T.astype(np.float32) / GS
    return out, res


def kernel(obs_encoding, lane_encoding, same_obs_mask, W, b):
    out, _ = _run(
        np.asarray(lane_encoding, dtype=np.float32),
        np.asarray(W, dtype=np.float32),
        np.asarray(b, dtype=np.float32),
    )
    return out


# revision 17
# speedup vs baseline: 1.7618x; 1.1712x over previous
"""Trainium2 Bass kernel for AttentionalAggregation-style GNN pooling.

reference math:
    enc  = relu(lane_encoding @ W.T + b)            # [M=400000, 512]
    maxp = segment_max(enc, seg)                    # [N=25000, 512], 16 lanes/group
    avgp = segment_mean(enc, seg)                   # [N=25000, 512]
    out  = concat([maxp, avgp], axis=1)             # [N, 1024]

Strategy (8 NeuronCores, data-parallel over lanes; each core owns whole
groups). Streaming per-block structure (keeps every engine overlapped):
  - Host pre-transposes x -> XT [128, M] bf16; single-pass bf16 matmul
    (the 2e-2 gate leaves ~5x margin over bf16 rounding noise).
  - relu(u + b) fused into the PSUM->SBUF evacuation on ACT, bf16 out.
  - max pool (DVE-only capability): per-span TT halves-tree on packed bf16
    (packed TENSOR_TENSOR hits the 2x DVE mode; TENSOR_REDUCE never does).
  - sum pool: a fraction of spans fully on the GPSIMD add-tree; the rest
    split DVE (level 1) + GPSIMD (levels 2-4) to balance both engines.
  - Pooled outputs stay transposed bf16 [512, G]; host converts to fp32
    and applies the /16 mean divide.
"""
import sys

sys.path.insert(0, "/opt/trn_rl_repo")

import numpy as np
import ml_dtypes

import concourse.bass as bass
import concourse.bacc as bacc
import concourse.tile as tile
from concourse import mybir
from concourse.bass_utils import run_bass_kernel_spmd

N_CORES = 8
IN_DIM = 128
OUT_DIM = 512
N_OBS = 25000
M_LANES = 400000
GS = 16
M_C = M_LANES // N_CORES
G_C = N_OBS // N_CORES
N_CHUNK = OUT_DIM // 128
BLK = 1536

# --- engine-balance tunables -------------------------------------------------
# sum routing per span index: span % 10 < GPS_SUM_TENTHS -> GPSIMD
# strided-pair add-tree, else DVE reduce_sum
GPS_SUM_TENTHS = 7
# max pool: True = TT halves-tree (fast in isolation but degrades under
# SBUF contention), False = plain reduce (robust 1.13 ns/row)
MAX_TREE = False

_compiled = {}


def _blocks(m_c, blk):
    out = []
    s = 0
    while s < m_c:
        out.append((s, min(blk, m_c - s)))
        s += blk
    return out


def _build(m_c: int = M_C) -> bass.Bass:
    nc = bacc.Bacc(None, target_bir_lowering=False)
    f32 = mybir.dt.float32
    bf16 = mybir.dt.bfloat16
    g_c = m_c // GS
    gb_max = BLK // GS

    xth_d = nc.dram_tensor("xth", [IN_DIM, m_c], bf16, kind="ExternalInput")
    wth_d = nc.dram_tensor("wth", [IN_DIM, OUT_DIM], bf16, kind="ExternalInput")
    bsc_d = nc.dram_tensor("bsc", [128, N_CHUNK], f32, kind="ExternalInput")
    omax_d = nc.dram_tensor("omax", [OUT_DIM, g_c], bf16, kind="ExternalOutput")
    osum_d = nc.dram_tensor("osum", [OUT_DIM, g_c], bf16, kind="ExternalOutput")

    blocks = _blocks(m_c, BLK)
    nblk = len(blocks)

    with nc.allow_low_precision("pooled outputs are bf16 by design"), \
            tile.TileContext(nc) as tc:
        with (
            tc.tile_pool(name="singles", bufs=1) as singles,
            tc.tile_pool(name="xin", bufs=3) as xin,
            tc.tile_pool(name="rsb", bufs=3) as rsb,
            tc.tile_pool(name="mt", bufs=2) as mtp,
            tc.tile_pool(name="st", bufs=2) as stp,
            tc.tile_pool(name="acc", bufs=1) as accp,
            tc.tile_pool(name="psum", bufs=2, space="PSUM") as psum,
        ):
            wth_sb = singles.tile([IN_DIM, OUT_DIM], bf16)
            nc.sync.dma_start(out=wth_sb, in_=wth_d[:, :])
            bsc_sb = singles.tile([128, N_CHUNK], f32)
            nc.sync.dma_start(out=bsc_sb, in_=bsc_d[:, :])

            maxp_sb = accp.tile([128, N_CHUNK, g_c], bf16)
            sump_sb = accp.tile([128, N_CHUNK, g_c], bf16)

            warm_sb = singles.tile([128, 2], f32)
            nc.vector.memset(warm_sb, 0.0)
            nc.scalar.activation(
                out=warm_sb, in_=warm_sb,
                func=mybir.ActivationFunctionType.Relu, bias=0.0, scale=1.0,
            )

            flush_at = {nblk // 4, nblk // 2, (3 * nblk) // 4, nblk - 1}
            flush_from = 0

            for ib, (l0, lb) in enumerate(blocks):
                gb = lb // GS
                g0 = l0 // GS

                xth_sb = xin.tile([IN_DIM, BLK], bf16, tag="xth")
                nc.sync.dma_start(out=xth_sb[:, :lb], in_=xth_d[:, l0 : l0 + lb])

                r_sb = rsb.tile([128, N_CHUNK, BLK], bf16, tag="r")

                n_wave = (lb + 511) // 512
                for c in range(N_CHUNK):
                    enc_ps = psum.tile([128, BLK], f32, tag="enc")
                    for w in range(n_wave):
                        w0 = w * 512
                        lw = min(512, lb - w0)
                        nc.tensor.matmul(
                            enc_ps[:, w0 : w0 + lw],
                            wth_sb[:, c * 128 : (c + 1) * 128],
                            xth_sb[:, w0 : w0 + lw],
                            start=True, stop=True,
                        )

                    rc = r_sb[:, c, :]
                    nc.scalar.activation(
                        out=rc[0:128, 0:lb],
                        in_=enc_ps[:, 0:lb],
                        func=mybir.ActivationFunctionType.Relu,
                        bias=bsc_sb[:, c : c + 1],
                        scale=1.0,
                    )

                    r3 = rc[0:128, 0:lb].rearrange("p (g s) -> p g s", s=GS)
                    span = ib * N_CHUNK + c

                    # ---- max pool (DVE halves-tree, packed bf16 2x) ----
                    if MAX_TREE:
                        m8 = mtp.tile([128, gb_max, 8], bf16, tag="m8")
                        nc.vector.tensor_tensor(
                            out=m8[:, 0:gb, :],
                            in0=r3[:, :, 0:8], in1=r3[:, :, 8:16],
                            op=mybir.AluOpType.max,
                        )
                        m4 = mtp.tile([128, gb_max, 4], bf16, tag="m4")
                        nc.vector.tensor_tensor(
                            out=m4[:, 0:gb, :],
                            in0=m8[:, 0:gb, 0:4], in1=m8[:, 0:gb, 4:8],
                            op=mybir.AluOpType.max,
                        )
                        m2 = mtp.tile([128, gb_max, 2], bf16, tag="m2")
                        nc.vector.tensor_tensor(
                            out=m2[:, 0:gb, :],
                            in0=m4[:, 0:gb, 0:2], in1=m4[:, 0:gb, 2:4],
                            op=mybir.AluOpType.max,
                        )
                        nc.vector.tensor_tensor(
                            out=maxp_sb[:, c, g0 : g0 + gb],
                            in0=m2[:, 0:gb, 0], in1=m2[:, 0:gb, 1],
                            op=mybir.AluOpType.max,
                        )
                    else:
                        nc.vector.reduce_max(
                            out=maxp_sb[:, c, g0 : g0 + gb],
                            in_=r3, axis=mybir.AxisListType.X,
                        )

                    # ---- sum pool ----
                    if span % 10 < GPS_SUM_TENTHS:
                        # gpsimd strided-pair add-tree (locality suits Q7)
                        s8 = stp.tile([128, gb_max, 8], bf16, tag="s8")
                        nc.gpsimd.tensor_tensor(
                            out=s8[:, 0:gb, :],
                            in0=r3[:, :, 0::2], in1=r3[:, :, 1::2],
                            op=mybir.AluOpType.add,
                        )
                        s4 = stp.tile([128, gb_max, 4], bf16, tag="s4")
                        nc.gpsimd.tensor_tensor(
                            out=s4[:, 0:gb, :],
                            in0=s8[:, 0:gb, 0::2], in1=s8[:, 0:gb, 1::2],
                            op=mybir.AluOpType.add,
                        )
                        s2 = stp.tile([128, gb_max, 2], bf16, tag="s2")
                        nc.gpsimd.tensor_tensor(
                            out=s2[:, 0:gb, :],
                            in0=s4[:, 0:gb, 0::2], in1=s4[:, 0:gb, 1::2],
                            op=mybir.AluOpType.add,
                        )
                        nc.gpsimd.tensor_tensor(
                            out=sump_sb[:, c, g0 : g0 + gb],
                            in0=s2[:, 0:gb, 0], in1=s2[:, 0:gb, 1],
                            op=mybir.AluOpType.add,
                        )
                    else:
                        nc.vector.reduce_sum(
                            out=sump_sb[:, c, g0 : g0 + gb],
                            in_=r3, axis=mybir.AxisListType.X,
                        )

                if ib in flush_at:
                    done = g0 + gb
                    if done > flush_from:
                        r0, r1 = flush_from, done
                        flush_from = done
                        for c in range(N_CHUNK):
                            nc.sync.dma_start(
                                out=omax_d[c * 128 : (c + 1) * 128, r0:r1],
                                in_=maxp_sb[:, c, r0:r1],
                            )
                            nc.sync.dma_start(
                                out=osum_d[c * 128 : (c + 1) * 128, r0:r1],
                                in_=sump_sb[:, c, r0:r1],
                            )

    nc.compile()
    return nc


def _get_nc() -> bass.Bass:
    if "k" not in _compiled:
        _compiled["k"] = _build()
    return _compiled["k"]


def _host_prep(lane_encoding, W, b):
    bf = ml_dtypes.bfloat16
    xT = np.ascontiguousarray(lane_encoding.T)
    xh = xT.astype(bf)
    wh = np.ascontiguousarray(W.T).astype(bf)
    bsc = np.ascontiguousarray(b.reshape(N_CHUNK, 128).T.astype(np.float32))

    in_maps = []
    for c in range(N_CORES):
        sl = slice(c * M_C, (c + 1) * M_C)
        in_maps.append({
            "xth": np.ascontiguousarray(xh[:, sl]),
            "wth": wh, "bsc": bsc,
        })
    return in_maps


def _run(lane_encoding, W, b, trace: bool = False):
    nc = _get_nc()
    in_maps = _host_prep(lane_encoding, W, b)
    try:
        res = run_bass_kernel_spmd(
            nc, in_maps, core_ids=list(range(N_CORES)), trace=trace
        )
    except Exception:
        res = run_bass_kernel_spmd(
            nc, in_maps, core_ids=list(range(N_CORES)), trace=trace
        )
    out = np.empty((N_OBS, 2 * OUT_DIM), dtype=np.float32)
    for c in range(N_CORES):
        gsl = slice(c * G_C, (c + 1) * G_C)
        out[gsl, :OUT_DIM] = res.results[c]["omax"].T.astype(np.float32)
        out[gsl, OUT_DIM:] = res.results[c]["osum"].T.astype(np.float32) / GS
    return out, res


def kernel(obs_encoding, lane_encoding, same_obs_mask, W, b):
    out, _ = _run(
        np.asarray(lane_encoding, dtype=np.float32),
        np.asarray(W, dtype=np.float32),
        np.asarray(b, dtype=np.float32),
    )
    return out


# revision 18
# speedup vs baseline: 1.8718x; 1.0625x over previous
"""Original staged baseline (313 us): bf16x3 matmul, ACT evac with /16 fold,
DVE reduce pools, gpsimd add-chain offload. Kept as a known-good fallback."""
import sys

sys.path.insert(0, "/opt/trn_rl_repo")

import numpy as np
import ml_dtypes

import concourse.bass as bass
import concourse.bacc as bacc
import concourse.tile as tile
from concourse import mybir
from concourse.bass_utils import run_bass_kernel_spmd

N_CORES = 8
IN_DIM = 128
OUT_DIM = 512
N_OBS = 25000
M_LANES = 400000
GS = 16
M_C = M_LANES // N_CORES
G_C = N_OBS // N_CORES
N_CHUNK = OUT_DIM // 128
BLK = 2048

MODE = "bf16x3"
GPS_SUM_TENTHS = 7

_compiled = {}


def _build(mode: str) -> bass.Bass:
    nc = bacc.Bacc(None, target_bir_lowering=False)
    f32 = mybir.dt.float32

    bf16 = mybir.dt.bfloat16
    xth_d = nc.dram_tensor("xth", [IN_DIM, M_C], bf16, kind="ExternalInput")
    xtl_d = nc.dram_tensor("xtl", [IN_DIM, M_C], bf16, kind="ExternalInput")
    wth_d = nc.dram_tensor("wth", [IN_DIM, OUT_DIM], bf16, kind="ExternalInput")
    wtl_d = nc.dram_tensor("wtl", [IN_DIM, OUT_DIM], bf16, kind="ExternalInput")
    bsc_d = nc.dram_tensor("bsc", [128, N_CHUNK], f32, kind="ExternalInput")
    omax_d = nc.dram_tensor("omax", [OUT_DIM, G_C], f32, kind="ExternalOutput")
    osum_d = nc.dram_tensor("osum", [OUT_DIM, G_C], f32, kind="ExternalOutput")

    with tile.TileContext(nc) as tc:
        with (
            tc.tile_pool(name="singles", bufs=1) as singles,
            tc.tile_pool(name="xin", bufs=3) as xin,
            tc.tile_pool(name="rsb", bufs=4) as rsb,
            tc.tile_pool(name="acc", bufs=1) as accp,
            tc.tile_pool(name="gtmp", bufs=4) as gtmp,
            tc.tile_pool(name="psum", bufs=2, space="PSUM") as psum,
        ):
            wth_sb = singles.tile([IN_DIM, OUT_DIM], bf16)
            nc.sync.dma_start(out=wth_sb, in_=wth_d[:, :])
            wtl_sb = singles.tile([IN_DIM, OUT_DIM], bf16)
            nc.sync.dma_start(out=wtl_sb, in_=wtl_d[:, :])
            bsc_sb = singles.tile([128, N_CHUNK], f32)
            nc.sync.dma_start(out=bsc_sb, in_=bsc_d[:, :])

            maxp_sb = accp.tile([128, N_CHUNK, G_C], f32)
            sump_sb = accp.tile([128, N_CHUNK, G_C], f32)

            warm_sb = singles.tile([128, 2], f32)
            nc.vector.memset(warm_sb, 0.0)
            nc.scalar.activation(
                out=warm_sb, in_=warm_sb,
                func=mybir.ActivationFunctionType.Relu, bias=0.0, scale=1.0,
            )

            starts = [0, 512]
            while starts[-1] + BLK < M_C:
                starts.append(starts[-1] + BLK)
            blocks = [(s, min(s + BLK, M_C) - s if i == len(starts) - 1
                       else (starts[i + 1] - s))
                      for i, s in enumerate(starts)]
            blocks = [(s, min(e, M_C - s)) for s, e in blocks]
            flush_after = {blocks[min(k, len(blocks) - 1)][0]
                           for k in (7, 13, 19, 23, len(blocks) - 1)}
            flush_from = 0

            for ib, (l0, lb) in enumerate(blocks):
                gb = lb // GS
                g0 = l0 // GS

                xth_sb = xin.tile([IN_DIM, BLK], bf16, tag="xth")
                nc.sync.dma_start(out=xth_sb[:, :lb], in_=xth_d[:, l0 : l0 + lb])
                xtl_sb = xin.tile([IN_DIM, BLK], bf16, tag="xtl")
                nc.sync.dma_start(out=xtl_sb[:, :lb], in_=xtl_d[:, l0 : l0 + lb])

                n_wave = (lb + 511) // 512
                for c in range(N_CHUNK):
                    enc_ps = psum.tile([128, BLK], f32, tag="enc")
                    for w in range(n_wave):
                        w0 = w * 512
                        lw = min(512, lb - w0)
                        out_ap = enc_ps[:, w0 : w0 + lw]
                        nc.tensor.matmul(
                            out_ap,
                            wth_sb[:, c * 128 : (c + 1) * 128],
                            xth_sb[:, w0 : w0 + lw],
                            start=True, stop=False,
                        )
                        nc.tensor.matmul(
                            out_ap,
                            wtl_sb[:, c * 128 : (c + 1) * 128],
                            xth_sb[:, w0 : w0 + lw],
                            start=False, stop=False,
                        )
                        nc.tensor.matmul(
                            out_ap,
                            wth_sb[:, c * 128 : (c + 1) * 128],
                            xtl_sb[:, w0 : w0 + lw],
                            start=False, stop=True,
                        )

                    r_sb = rsb.tile([128, BLK], f32, tag="r")
                    nc.scalar.activation(
                        out=r_sb[:, :lb],
                        in_=enc_ps[:, :lb],
                        func=mybir.ActivationFunctionType.Relu,
                        bias=bsc_sb[:, c : c + 1],
                        scale=1.0 / GS,
                    )
                    r3 = r_sb[:, :lb].rearrange("p (g s) -> p g s", s=GS)
                    nc.vector.reduce_max(
                        out=maxp_sb[:, c, g0 : g0 + gb],
                        in_=r3,
                        axis=mybir.AxisListType.X,
                    )
                    span_idx = ib * N_CHUNK + c
                    if span_idx % 10 < GPS_SUM_TENTHS:
                        t1 = gtmp.tile([128, gb, 8], f32, tag="t1")
                        nc.gpsimd.tensor_tensor(
                            out=t1, in0=r3[:, :, 0::2], in1=r3[:, :, 1::2],
                            op=mybir.AluOpType.add,
                        )
                        t2 = gtmp.tile([128, gb, 4], f32, tag="t2")
                        nc.gpsimd.tensor_tensor(
                            out=t2, in0=t1[:, :, 0::2], in1=t1[:, :, 1::2],
                            op=mybir.AluOpType.add,
                        )
                        t3 = gtmp.tile([128, gb, 2], f32, tag="t3")
                        nc.gpsimd.tensor_tensor(
                            out=t3, in0=t2[:, :, 0::2], in1=t2[:, :, 1::2],
                            op=mybir.AluOpType.add,
                        )
                        nc.gpsimd.tensor_tensor(
                            out=sump_sb[:, c, g0 : g0 + gb],
                            in0=t3[:, :, 0], in1=t3[:, :, 1],
                            op=mybir.AluOpType.add,
                        )
                    else:
                        nc.vector.reduce_sum(
                            out=sump_sb[:, c, g0 : g0 + gb],
                            in_=r3,
                            axis=mybir.AxisListType.X,
                        )

                if l0 in flush_after:
                    r0, r1 = flush_from, g0 + gb
                    flush_from = r1
                    nc.scalar.mul(
                        out=maxp_sb[:, :, r0:r1],
                        in_=maxp_sb[:, :, r0:r1],
                        mul=float(GS),
                    )
                    for c in range(N_CHUNK):
                        nc.sync.dma_start(
                            out=omax_d[c * 128 : (c + 1) * 128, r0:r1],
                            in_=maxp_sb[:, c, r0:r1],
                        )
                        nc.sync.dma_start(
                            out=osum_d[c * 128 : (c + 1) * 128, r0:r1],
                            in_=sump_sb[:, c, r0:r1],
                        )

    nc.compile()
    return nc


def _get_nc(mode: str) -> bass.Bass:
    if mode not in _compiled:
        _compiled[mode] = _build(mode)
    return _compiled[mode]


def _host_prep(lane_encoding, W, b, mode: str):
    xT = np.ascontiguousarray(lane_encoding.T)
    wT = np.ascontiguousarray(W.T)
    bsc = np.ascontiguousarray(
        (b.reshape(N_CHUNK, 128).T / GS).astype(np.float32)
    )

    in_maps = []
    bf = ml_dtypes.bfloat16
    xh = xT.astype(bf)
    xl = (xT - xh.astype(np.float32)).astype(bf)
    wh = wT.astype(bf)
    wl = (wT - wh.astype(np.float32)).astype(bf)
    for c in range(N_CORES):
        sl = slice(c * M_C, (c + 1) * M_C)
        in_maps.append({
            "xth": np.ascontiguousarray(xh[:, sl]),
            "xtl": np.ascontiguousarray(xl[:, sl]),
            "wth": wh, "wtl": wl, "bsc": bsc,
        })
    return in_maps


def _run(lane_encoding, W, b, mode: str = MODE, trace: bool = False):
    nc = _get_nc(mode)
    in_maps = _host_prep(lane_encoding, W, b, mode)
    try:
        res = run_bass_kernel_spmd(
            nc, in_maps, core_ids=list(range(N_CORES)), trace=trace
        )
    except Exception:
        res = run_bass_kernel_spmd(
            nc, in_maps, core_ids=list(range(N_CORES)), trace=trace
        )
    out = np.empty((N_OBS, 2 * OUT_DIM), dtype=np.float32)
    for c in range(N_CORES):
        gsl = slice(c * G_C, (c + 1) * G_C)
        out[gsl, :OUT_DIM] = res.results[c]["omax"].T
        out[gsl, OUT_DIM:] = res.results[c]["osum"].T
    return out, res


def kernel(obs_encoding, lane_encoding, same_obs_mask, W, b):
    out, _ = _run(
        np.asarray(lane_encoding, dtype=np.float32),
        np.asarray(W, dtype=np.float32),
        np.asarray(b, dtype=np.float32),
        MODE,
    )
    return out


# revision 25
# speedup vs baseline: 1.9130x; 1.0220x over previous
"""Baseline structure (streaming blocks, ACT evac with /16 fold, DVE reduce
pools, gpsimd add-chain offload at 70%) with a single-pass bf16 matmul:
the 2e-2 rel-err gate leaves ~5x margin over bf16 rounding noise, and one
matmul pass cuts Tensor-engine busy from ~261 us to ~100-170 us and input
DMA in half. Pool/evac path is identical to the measured-optimal baseline."""
import sys

sys.path.insert(0, "/opt/trn_rl_repo")

import numpy as np
import ml_dtypes

import concourse.bass as bass
import concourse.bacc as bacc
import concourse.tile as tile
from concourse import mybir
from concourse.bass_utils import run_bass_kernel_spmd

N_CORES = 8
IN_DIM = 128
OUT_DIM = 512
N_OBS = 25000
M_LANES = 400000
GS = 16
M_C = M_LANES // N_CORES
G_C = N_OBS // N_CORES
N_CHUNK = OUT_DIM // 128
BLK = 2048

MODE = "bf16x1"
GPS_SUM_TENTHS = 7

_compiled = {}


def _build(mode: str) -> bass.Bass:
    nc = bacc.Bacc(None, target_bir_lowering=False)
    f32 = mybir.dt.float32

    bf16 = mybir.dt.bfloat16
    xth_d = nc.dram_tensor("xth", [IN_DIM, M_C], bf16, kind="ExternalInput")
    wth_d = nc.dram_tensor("wth", [IN_DIM, OUT_DIM], bf16, kind="ExternalInput")
    bsc_d = nc.dram_tensor("bsc", [128, N_CHUNK], f32, kind="ExternalInput")
    omax_d = nc.dram_tensor("omax", [OUT_DIM, G_C], f32, kind="ExternalOutput")
    osum_d = nc.dram_tensor("osum", [OUT_DIM, G_C], f32, kind="ExternalOutput")

    with tile.TileContext(nc) as tc:
        with (
            tc.tile_pool(name="singles", bufs=1) as singles,
            tc.tile_pool(name="xin", bufs=3) as xin,
            tc.tile_pool(name="rsb", bufs=4) as rsb,
            tc.tile_pool(name="acc", bufs=1) as accp,
            tc.tile_pool(name="gtmp", bufs=4) as gtmp,
            tc.tile_pool(name="psum", bufs=2, space="PSUM") as psum,
        ):
            wth_sb = singles.tile([IN_DIM, OUT_DIM], bf16)
            nc.sync.dma_start(out=wth_sb, in_=wth_d[:, :])
            bsc_sb = singles.tile([128, N_CHUNK], f32)
            nc.sync.dma_start(out=bsc_sb, in_=bsc_d[:, :])

            maxp_sb = accp.tile([128, N_CHUNK, G_C], f32)
            sump_sb = accp.tile([128, N_CHUNK, G_C], f32)

            warm_sb = singles.tile([128, 2], f32)
            nc.vector.memset(warm_sb, 0.0)
            nc.scalar.activation(
                out=warm_sb, in_=warm_sb,
                func=mybir.ActivationFunctionType.Relu, bias=0.0, scale=1.0,
            )

            starts = [0, 512]
            while starts[-1] + BLK < M_C:
                starts.append(starts[-1] + BLK)
            blocks = [(s, min(s + BLK, M_C) - s if i == len(starts) - 1
                       else (starts[i + 1] - s))
                      for i, s in enumerate(starts)]
            blocks = [(s, min(e, M_C - s)) for s, e in blocks]
            flush_after = {blocks[min(k, len(blocks) - 1)][0]
                           for k in (7, 13, 19, 23, len(blocks) - 1)}
            flush_from = 0

            for ib, (l0, lb) in enumerate(blocks):
                gb = lb // GS
                g0 = l0 // GS

                xth_sb = xin.tile([IN_DIM, BLK], bf16, tag="xth")
                nc.sync.dma_start(out=xth_sb[:, :lb], in_=xth_d[:, l0 : l0 + lb])

                n_wave = (lb + 511) // 512
                for c in range(N_CHUNK):
                    enc_ps = psum.tile([128, BLK], f32, tag="enc")
                    for w in range(n_wave):
                        w0 = w * 512
                        lw = min(512, lb - w0)
                        out_ap = enc_ps[:, w0 : w0 + lw]
                        nc.tensor.matmul(
                            out_ap,
                            wth_sb[:, c * 128 : (c + 1) * 128],
                            xth_sb[:, w0 : w0 + lw],
                            start=True, stop=True,
                        )

                    r_sb = rsb.tile([128, BLK], f32, tag="r")
                    nc.scalar.activation(
                        out=r_sb[:, :lb],
                        in_=enc_ps[:, :lb],
                        func=mybir.ActivationFunctionType.Relu,
                        bias=bsc_sb[:, c : c + 1],
                        scale=1.0 / GS,
                    )
                    r3 = r_sb[:, :lb].rearrange("p (g s) -> p g s", s=GS)
                    nc.vector.reduce_max(
                        out=maxp_sb[:, c, g0 : g0 + gb],
                        in_=r3,
                        axis=mybir.AxisListType.X,
                    )
                    span_idx = ib * N_CHUNK + c
                    if span_idx % 10 < GPS_SUM_TENTHS:
                        t1 = gtmp.tile([128, gb, 8], f32, tag="t1")
                        nc.gpsimd.tensor_tensor(
                            out=t1, in0=r3[:, :, 0::2], in1=r3[:, :, 1::2],
                            op=mybir.AluOpType.add,
                        )
                        t2 = gtmp.tile([128, gb, 4], f32, tag="t2")
                        nc.gpsimd.tensor_tensor(
                            out=t2, in0=t1[:, :, 0::2], in1=t1[:, :, 1::2],
                            op=mybir.AluOpType.add,
                        )
                        t3 = gtmp.tile([128, gb, 2], f32, tag="t3")
                        nc.gpsimd.tensor_tensor(
                            out=t3, in0=t2[:, :, 0::2], in1=t2[:, :, 1::2],
                            op=mybir.AluOpType.add,
                        )
                        nc.gpsimd.tensor_tensor(
                            out=sump_sb[:, c, g0 : g0 + gb],
                            in0=t3[:, :, 0], in1=t3[:, :, 1],
                            op=mybir.AluOpType.add,
                        )
                    else:
                        nc.vector.reduce_sum(
                            out=sump_sb[:, c, g0 : g0 + gb],
                            in_=r3,
                            axis=mybir.AxisListType.X,
                        )

                if l0 in flush_after:
                    r0, r1 = flush_from, g0 + gb
                    flush_from = r1
                    nc.scalar.mul(
                        out=maxp_sb[:, :, r0:r1],
                        in_=maxp_sb[:, :, r0:r1],
                        mul=float(GS),
                    )
                    for c in range(N_CHUNK):
                        nc.sync.dma_start(
                            out=omax_d[c * 128 : (c + 1) * 128, r0:r1],
                            in_=maxp_sb[:, c, r0:r1],
                        )
                        nc.sync.dma_start(
                            out=osum_d[c * 128 : (c + 1) * 128, r0:r1],
                            in_=sump_sb[:, c, r0:r1],
                        )

    nc.compile()
    return nc


def _get_nc(mode: str) -> bass.Bass:
    if mode not in _compiled:
        _compiled[mode] = _build(mode)
    return _compiled[mode]


def _host_prep(lane_encoding, W, b, mode: str):
    xT = np.ascontiguousarray(lane_encoding.T)
    wT = np.ascontiguousarray(W.T)
    bsc = np.ascontiguousarray(
        (b.reshape(N_CHUNK, 128).T / GS).astype(np.float32)
    )

    in_maps = []
    bf = ml_dtypes.bfloat16
    xh = xT.astype(bf)
    wh = wT.astype(bf)
    for c in range(N_CORES):
        sl = slice(c * M_C, (c + 1) * M_C)
        in_maps.append({
            "xth": np.ascontiguousarray(xh[:, sl]),
            "wth": wh, "bsc": bsc,
        })
    return in_maps


def _run(lane_encoding, W, b, mode: str = MODE, trace: bool = False):
    nc = _get_nc(mode)
    in_maps = _host_prep(lane_encoding, W, b, mode)
    try:
        res = run_bass_kernel_spmd(
            nc, in_maps, core_ids=list(range(N_CORES)), trace=trace
        )
    except Exception:
        res = run_bass_kernel_spmd(
            nc, in_maps, core_ids=list(range(N_CORES)), trace=trace
        )
    out = np.empty((N_OBS, 2 * OUT_DIM), dtype=np.float32)
    for c in range(N_CORES):
        gsl = slice(c * G_C, (c + 1) * G_C)
        out[gsl, :OUT_DIM] = res.results[c]["omax"].T
        out[gsl, OUT_DIM:] = res.results[c]["osum"].T
    return out, res


def kernel(obs_encoding, lane_encoding, same_obs_mask, W, b):
    out, _ = _run(
        np.asarray(lane_encoding, dtype=np.float32),
        np.asarray(W, dtype=np.float32),
        np.asarray(b, dtype=np.float32),
        MODE,
    )
    return out
